# revision 1
# baseline (speedup 1.0000x reference)
"""Trainium2 Bass kernel for the fixed expression tree:

    l1 = x @ c1; l2 = x @ c2
    u1 = w1*sin(l1)+b1; u2 = w2*relu(l2)+b2
    y  = wr*tanh(u1*u2)+br

x is [131072, 1024] fp32. Data-parallel over 8 NeuronCores: each core gets
16384 rows; the tiny coefficients/scalars are replicated. No communication.

Per-core pipeline (all weights/data in float32r — same fp32 bits, PE rounds
operands to ~13 mantissa bits but runs matmuls at 1 cyc/row instead of 4):
  1. DMA a 4 MiB panel (1024 rows) of x into SBUF in natural [row, d] layout.
  2. PE transposes each [128 rows x 128 d] block (matmul vs identity) into
     PSUM; DVE/ACT copy the transposed blocks back to SBUF. This puts the
     contraction dim (d) on partitions.
  3. PE dot matmuls: lhsT = [c1|c2] block [128, 2], rhs = xT [128, 512 rows],
     accumulated over the 8 d-blocks -> psum l [2, 512].
  4. PE mini-transposes flip l [2, 128] slices into [128, 2] so rows land on
     partitions; results accumulate into an SBUF staging buffer.
  5. One fused elementwise epilogue over all 16384 rows at once (ACT for
     sin/relu/tanh, DVE for the affine/multiply ops), final PE transpose to
     restore row-major order, DMA out.
"""

import numpy as np

N_CORES = 8
B = 131072
D = 1024
R_CORE = B // N_CORES  # 16384 rows per core
PANEL_T = 8  # 128-row tiles per panel
PANEL_ROWS = 128 * PANEL_T  # 1024
N_PANELS = R_CORE // PANEL_ROWS  # 16

_cache = {}


def _build():
    import concourse.bass as bass
    import concourse.tile as tile
    from concourse import bacc, mybir
    from concourse.masks import make_identity

    FP32 = mybir.dt.float32
    F32R = mybir.dt.float32r
    AOT = mybir.ActivationFunctionType
    ALU = mybir.AluOpType

    nc = bacc.Bacc("TRN2", target_bir_lowering=False, debug=False, num_devices=N_CORES)
    x_d = nc.dram_tensor("x", [R_CORE, D], F32R, kind="ExternalInput").ap()
    c1_d = nc.dram_tensor("c1", [D], F32R, kind="ExternalInput").ap()
    c2_d = nc.dram_tensor("c2", [D], F32R, kind="ExternalInput").ap()
    p_d = nc.dram_tensor("p", [6], FP32, kind="ExternalInput").ap()
    y_d = nc.dram_tensor("y", [R_CORE], F32R, kind="ExternalOutput").ap()

    xr = x_d.rearrange("(n t p) d -> n p t d", p=128, t=PANEL_T)
    yr = y_d.rearrange("(c q) -> c q", q=128)  # [128 tiles, 128 rows-in-tile]

    with tile.TileContext(nc) as tc:
        with (
            tc.tile_pool(name="singles", bufs=1) as singles,
            tc.tile_pool(name="xp", bufs=3) as xp,
            tc.tile_pool(name="xtp", bufs=2) as xtp,
            tc.tile_pool(name="small", bufs=3) as small,
            tc.tile_pool(name="pst", bufs=3, space="PSUM") as pst,
            tc.tile_pool(name="psl", bufs=2, space="PSUM") as psl,
            tc.tile_pool(name="pse", bufs=2, space="PSUM") as pse,
        ):
            ident_f = singles.tile([128, 128], FP32)
            make_identity(nc, ident_f)
            ident = singles.tile([128, 128], F32R)
            nc.vector.tensor_copy(out=ident, in_=ident_f)

            # coefficient pairs: c_sb[p, v, j] = c_v[128*j + p]
            c_sb = singles.tile([128, 2, 8], F32R)
            nc.sync.dma_start(out=c_sb[:, 0, :], in_=c1_d.rearrange("(j p) -> p j", p=128))
            nc.sync.dma_start(out=c_sb[:, 1, :], in_=c2_d.rearrange("(j p) -> p j", p=128))

            # scalars broadcast to all partitions: w_sb[:, i] = p[i]
            w_sb = singles.tile([128, 6], FP32)
            nc.gpsimd.dma_start(out=w_sb, in_=p_d.partition_broadcast(128))

            # l staging: sb_y[q, 16*n + 2*k + v] = l_v[row 1024*n + 128*k + q]
            sb_y = singles.tile([128, 2 * (R_CORE // 128)], FP32)

            for n in range(N_PANELS):
                x_nat = xp.tile([128, PANEL_T, D], F32R, tag="x")
                nc.sync.dma_start(out=x_nat[:, 0 : PANEL_T // 2, :], in_=xr[n][:, 0 : PANEL_T // 2, :])
                nc.sync.dma_start(out=x_nat[:, PANEL_T // 2 :, :], in_=xr[n][:, PANEL_T // 2 :, :])

                # transpose all 64 [128,128] blocks of the panel; sb_xt is
                # [dj, j, t*128+q] so each j-slice is a [128, 1024] rhs.
                sb_xt = xtp.tile([128, 8, PANEL_ROWS], F32R, tag="xt")
                for j in range(8):
                    for h in range(PANEL_T // 4):
                        pt = pst.tile([128, 512], F32R, tag="pt")
                        for tt in range(4):
                            t = 4 * h + tt
                            nc.tensor.transpose(
                                pt[:, 128 * tt : 128 * (tt + 1)],
                                x_nat[:, t, 128 * j : 128 * (j + 1)],
                                ident,
                            )
                        # alternate copy engine to split the PSUM->SBUF load
                        if (j + h) % 2 == 0:
                            nc.vector.tensor_copy(
                                out=sb_xt[:, j, 512 * h : 512 * (h + 1)], in_=pt
                            )
                        else:
                            nc.scalar.copy(
                                out=sb_xt[:, j, 512 * h : 512 * (h + 1)], in_=pt
                            )

                # dot products: accumulate over j into l [2, 512] per half-panel
                sb_l = small.tile([2, PANEL_ROWS], F32R, tag="l")
                for half in range(PANEL_ROWS // 512):
                    ps_l = psl.tile([2, 512], FP32, tag="psl")
                    for j in range(8):
                        nc.tensor.matmul(
                            ps_l,
                            c_sb[:, :, j],
                            sb_xt[:, j, 512 * half : 512 * (half + 1)],
                            start=(j == 0),
                            stop=(j == 7),
                        )
                    if half % 2 == 0:
                        nc.vector.tensor_copy(
                            out=sb_l[:, 512 * half : 512 * (half + 1)], in_=ps_l
                        )
                    else:
                        nc.scalar.copy(
                            out=sb_l[:, 512 * half : 512 * (half + 1)], in_=ps_l
                        )

                # flip rows onto partitions: [2, 128] -> [128, 2] per k
                ps_e = pse.tile([128, 2 * PANEL_T], F32R, tag="pse")
                for k in range(PANEL_T):
                    nc.tensor.transpose(
                        ps_e[:, 2 * k : 2 * (k + 1)],
                        sb_l[:, 128 * k : 128 * (k + 1)],
                        ident[0:2, 0:2],
                    )
                nc.vector.tensor_copy(
                    out=sb_y[:, 2 * PANEL_T * n : 2 * PANEL_T * (n + 1)], in_=ps_e
                )

            # ---- fused elementwise epilogue over all rows ----
            NT = R_CORE // 128  # 128 l-columns
            l1 = sb_y.rearrange("q (c v) -> q c v", v=2)[:, :, 0]
            l2 = sb_y.rearrange("q (c v) -> q c v", v=2)[:, :, 1]
            # range-reduce l1 into [-pi, pi] before Sin: the ACT Sin LUT is
            # only accurate for |x| < ~3.95 and |l1| reaches ~4.9.
            INV2PI = 0.15915494309189535
            TWOPI = 6.283185307179586
            MAGIC = 12582912.0  # 1.5 * 2**23: (t + M) - M rounds t to nearest int
            kk = small.tile([128, NT], FP32, tag="e0a")
            nc.vector.tensor_scalar(
                out=kk, in0=l1,
                scalar1=INV2PI, scalar2=MAGIC,
                op0=ALU.mult, op1=ALU.add,
            )
            kred = small.tile([128, NT], FP32, tag="e0b")
            nc.vector.tensor_scalar(
                out=kred, in0=kk,
                scalar1=-MAGIC, scalar2=-TWOPI,
                op0=ALU.add, op1=ALU.mult,
            )
            lred = small.tile([128, NT], FP32, tag="e0c")
            nc.vector.tensor_add(out=lred, in0=l1, in1=kred)
            s1 = small.tile([128, NT], FP32, tag="e1")
            nc.scalar.activation(out=s1, in_=lred, func=AOT.Sin)
            u1 = small.tile([128, NT], FP32, tag="e2")
            nc.vector.tensor_scalar(
                out=u1, in0=s1,
                scalar1=w_sb[:, 0:1], scalar2=w_sb[:, 1:2],
                op0=ALU.mult, op1=ALU.add,
            )
            r2 = small.tile([128, NT], FP32, tag="e3")
            nc.scalar.activation(out=r2, in_=l2, func=AOT.Relu)
            u2 = small.tile([128, NT], FP32, tag="e4")
            nc.vector.tensor_scalar(
                out=u2, in0=r2,
                scalar1=w_sb[:, 2:3], scalar2=w_sb[:, 3:4],
                op0=ALU.mult, op1=ALU.add,
            )
            v = small.tile([128, NT], FP32, tag="e5")
            nc.vector.tensor_mul(out=v, in0=u1, in1=u2)
            th = small.tile([128, NT], FP32, tag="e6")
            nc.scalar.activation(out=th, in_=v, func=AOT.Tanh)
            y_sb = small.tile([128, NT], F32R, tag="e7")
            nc.vector.tensor_scalar(
                out=y_sb, in0=th,
                scalar1=w_sb[:, 4:5], scalar2=w_sb[:, 5:6],
                op0=ALU.mult, op1=ALU.add,
            )

            # final transpose to row-major and store
            ps_y = pse.tile([128, 128], F32R, tag="psy", bufs=1)
            nc.tensor.transpose(ps_y, y_sb, ident)
            yt = small.tile([128, 128], F32R, tag="yt")
            nc.vector.tensor_copy(out=yt, in_=ps_y)
            nc.sync.dma_start(out=yr, in_=yt)

    nc.compile()
    return nc


def _get_nc():
    if "nc" not in _cache:
        _cache["nc"] = _build()
    return _cache["nc"]


def _execute(inputs, trace=False):
    from concourse.bass_utils import run_bass_kernel_spmd

    nc = _get_nc()
    x = np.ascontiguousarray(np.asarray(inputs["x"], dtype=np.float32))
    c1 = np.ascontiguousarray(np.asarray(inputs["c1"], dtype=np.float32))
    c2 = np.ascontiguousarray(np.asarray(inputs["c2"], dtype=np.float32))
    p = np.stack(
        [
            np.float32(np.asarray(inputs[k]).reshape(()))
            for k in ("w1", "b1", "w2", "b2", "wr", "br")
        ]
    ).astype(np.float32)
    in_maps = [
        {
            "x": x[i * R_CORE : (i + 1) * R_CORE],
            "c1": c1,
            "c2": c2,
            "p": p,
        }
        for i in range(N_CORES)
    ]
    res = run_bass_kernel_spmd(
        nc, in_maps, core_ids=list(range(N_CORES)), trace=trace
    )
    y = np.concatenate([res.results[i]["y"] for i in range(N_CORES)])
    return y.astype(np.float32), res


def kernel(**inputs) -> np.ndarray:
    y, _ = _execute(inputs, trace=False)
    return y

